# revision 1
# baseline (speedup 1.0000x reference)
"""Trainium2 Bass kernel for nn_Attn_55611236548746.

Attention pooling:
    energies[b,t] = enc[b,t,:]@w_e + hid_flat[b,:]@w_h + bias
    p = renorm(mask * softmax(energies * mask))
    out[b,:]     = sum_t p[b,t] * enc[b,t,:]

Sharding: data-parallel over B (32 batches -> 4 per core on 8 cores);
attn weights replicated.

Per-core design (memory-bound; HBM floor = 32MB / ~360GB/s ~ 89us):
  - encoder tiles (128t x 1024e) f32 stream into SBUF once (t on partitions),
    double-buffered per batch (2 x 8MB).
  - energies: fused DVE tensor_tensor_reduce (mult + row-sum) per tile,
    seeded with the per-batch hidden scalar -> en (128,16).
  - Exact softmax algebra: the first softmax's denominator cancels with the
    final renormalization, so  p_t = mask_t*exp(mask_t*en_t) / sum(...).
    No max subtraction needed (|logits| < ~8 for this data scale).
  - weighted pool: PE matmuls contracting over t (u column as lhsT),
    accumulated in PSUM; final scale by 1/sum(u) on ScalarE.
"""

import numpy as np

N_CORES = 8
B, T, E = 32, 2048, 1024
LD, HD = 2, 1024          # hidden: (LD, B, HD)
DEC = LD * HD             # 2048 = flattened-hidden width
BP = B // N_CORES         # 4 batches per core
TB = T // 128             # 16 t-blocks of 128

_nc_cache = {}


def _build(reps=1, body_mult=1, mode="full"):
    """reps>1 wraps the main loop in a hardware For_i for benchmarking;
    body_mult repeats the whole 4-batch body inside the loop.
    mode: full | dma (loads only) | dve (loads+energies) — bench variants."""
    from contextlib import ExitStack

    import concourse.bacc as bacc
    import concourse.tile as tile
    from concourse import mybir
    from concourse._compat import with_exitstack
    from concourse.alu_op_type import AluOpType

    f32 = mybir.dt.float32
    f32r = mybir.dt.float32r
    MUL, ADD = AluOpType.mult, AluOpType.add
    EXP = mybir.ActivationFunctionType.Exp
    COPY = mybir.ActivationFunctionType.Copy
    IDENT = mybir.ActivationFunctionType.Identity

    nc = bacc.Bacc("TRN2", target_bir_lowering=False, debug=False,
                   num_devices=N_CORES)
    enc = nc.dram_tensor("enc", [BP, T, E], f32, kind="ExternalInput").ap()
    hid = nc.dram_tensor("hid", [LD, BP, HD], f32, kind="ExternalInput").ap()
    msk = nc.dram_tensor("msk", [BP, T], f32, kind="ExternalInput").ap()
    w = nc.dram_tensor("w", [DEC + E], f32, kind="ExternalInput").ap()
    bia = nc.dram_tensor("bia", [1], f32, kind="ExternalInput").ap()
    out = nc.dram_tensor("out", [BP, E], f32, kind="ExternalOutput").ap()

    @with_exitstack
    def body(ctx, tc):
        consts = ctx.enter_context(tc.tile_pool(name="consts", bufs=1))
        encp = ctx.enter_context(tc.tile_pool(name="encp", bufs=2))
        scrp = ctx.enter_context(tc.tile_pool(name="scrp", bufs=3))
        small = ctx.enter_context(tc.tile_pool(name="small", bufs=3))
        outp = ctx.enter_context(tc.tile_pool(name="outp", bufs=2))
        pso = ctx.enter_context(tc.tile_pool(name="pso", bufs=2, space="PSUM"))
        pst = ctx.enter_context(tc.tile_pool(name="pst", bufs=2, space="PSUM"))

        # ---- constants / per-core preamble ----
        w_bc = consts.tile([128, DEC + E], f32)
        nc.gpsimd.dma_start(out=w_bc, in_=w[None, :].to_broadcast([128, DEC + E]))
        mask_sb = consts.tile([128, BP, TB], f32)
        nc.gpsimd.dma_start(out=mask_sb, in_=msk.rearrange("b (p j) -> p b j", p=128))
        hid_sb = consts.tile([BP, LD, HD], f32)
        nc.gpsimd.dma_start(out=hid_sb, in_=hid.rearrange("l b e -> b l e"))
        b_bc = consts.tile([BP, 1], f32)
        nc.gpsimd.dma_start(out=b_bc, in_=bia[None, :].to_broadcast([BP, 1]))
        ones_col = consts.tile([128, 1], f32)
        nc.vector.memset(ones_col, 1.0)
        ones_row = consts.tile([1, 128], f32)
        nc.vector.memset(ones_row, 1.0)

        # h[b] = hid_flat[b] . w_h, then broadcast to all partitions:
        # (4,1) column -> 32x32 DVE transpose -> (1,4) row -> k=1 outer-product
        # matmul with a ones row -> (128,4) in PSUM -> SBUF (+ bias via the
        # activation's per-partition bias input).
        b_bc128 = consts.tile([128, 1], f32)
        nc.gpsimd.dma_start(out=b_bc128, in_=bia[None, :].to_broadcast([128, 1]))
        h32 = consts.tile([32, 32], f32)
        nc.vector.memset(h32, 0.0)
        hscr = consts.tile([BP, DEC], f32)
        nc.vector.scalar_tensor_tensor(
            out=hscr, in0=hid_sb.rearrange("b l e -> b (l e)"), scalar=0.0,
            in1=w_bc[0:BP, 0:DEC], op0=ADD, op1=MUL,
            accum_out=h32[0:BP, 0:1])
        h32t = consts.tile([32, 32], f32)
        nc.vector.transpose(out=h32t, in_=h32)
        h_ps = pst.tile([128, BP], f32)
        nc.tensor.matmul(h_ps, ones_row, h32t[0:1, 0:BP], start=True, stop=True)
        h_bc = consts.tile([128, BP], f32)
        nc.scalar.activation(out=h_bc, in_=h_ps, func=IDENT, bias=b_bc128,
                             scale=1.0)

        # ---- main loop over this core's batches ----
        def main_loop():
            for b in range(BP):
                # SWDGE casting DMA: f32 HBM -> f32r SBUF (rounds for the PE's
                # full-rate f32r matmul path; DVE reads the f32 bitcast view).
                # contiguous per-partition layout: t = 16*p + j, so each
                # partition reads one 32KB contiguous HBM run per half.
                enc_sb = encp.tile([128, TB, E], f32r)
                encb = enc[b].rearrange("(p j) e -> p j e", p=128)
                for c in range(2):
                    nc.gpsimd.dma_start(out=enc_sb[:, 8 * c:8 * (c + 1), :],
                                        in_=encb[:, 8 * c:8 * (c + 1), :])

                if mode == "dma":
                    sink = small.tile([1, 16], f32)
                    nc.vector.tensor_copy(sink, enc_sb[0:1, 0, 0:16].bitcast(f32))
                    continue

                # energies: en[:, i] = enc_tile_i @ w_e  (fused mult + row-sum)
                en = small.tile([128, TB], f32)
                for i in range(TB):
                    s = scrp.tile([128, E], f32)
                    nc.vector.scalar_tensor_tensor(
                        out=s, in0=enc_sb[:, i, :].bitcast(f32), scalar=0.0,
                        in1=w_bc[:, DEC:DEC + E], op0=ADD, op1=MUL,
                        accum_out=en[:, i:i + 1])

                if mode == "dve":
                    sink = small.tile([1, 16], f32)
                    nc.vector.tensor_copy(sink, en[0:1, :])
                    continue

                # u = mask * exp((en + h[b]) * mask);  us = row-sums of u
                x2 = small.tile([128, TB], f32)
                nc.vector.scalar_tensor_tensor(
                    out=x2, in0=en, scalar=h_bc[:, b:b + 1],
                    in1=mask_sb[:, b, :], op0=ADD, op1=MUL)
                u0 = small.tile([128, TB], f32)
                nc.scalar.activation(out=u0, in_=x2, func=EXP)
                u = small.tile([128, TB], f32)
                us = small.tile([128, 1], f32)
                nc.vector.scalar_tensor_tensor(
                    out=u, in0=u0, scalar=0.0, in1=mask_sb[:, b, :],
                    op0=ADD, op1=MUL, accum_out=us)

                ur = small.tile([128, TB], f32r)
                nc.scalar.copy(out=ur, in_=u)

                # total = sum_p us ; rt = 1/total
                tot = pst.tile([1, 1], f32)
                nc.tensor.matmul(tot, us, ones_col, start=True, stop=True)
                rt = small.tile([1, 1], f32)
                nc.vector.reciprocal(out=rt, in_=tot)

                # weighted pool: po[0, e] = sum_t u[t] * enc[t, e]
                # f32r: full-rate PE (1 cyc/col vs 4 for fp32), ~19-bit multiply
                # precision with fp32 PSUM accumulate — well inside tolerance.
                po = pso.tile([1, E], f32)
                for half in range(2):
                    sl = slice(half * 512, (half + 1) * 512)
                    for i in range(TB):
                        nc.tensor.matmul(po[:, sl], ur[:, i:i + 1],
                                         enc_sb[:, i, sl],
                                         start=(i == 0), stop=(i == TB - 1))

                ob = outp.tile([1, E], f32)
                nc.scalar.activation(out=ob, in_=po, func=COPY, scale=rt)
                nc.scalar.dma_start(out=out[b], in_=ob)

        if reps == 1:
            for _ in range(body_mult):
                main_loop()
        else:
            with tc.For_i(0, reps, 1):
                for _ in range(body_mult):
                    main_loop()

    with tile.TileContext(nc) as tc:
        body(tc)
    nc.compile()
    return nc


def _get_nc(reps=1, body_mult=1, mode="full"):
    key = (reps, body_mult, mode)
    if key not in _nc_cache:
        _nc_cache[key] = _build(reps, body_mult, mode)
    return _nc_cache[key]


def _run(hidden, encoder_outputs, mask, attn_w, attn_b, trace=False,
         trace_kwargs=None, reps=1, body_mult=1, mode="full"):
    from concourse.bass_utils import run_bass_kernel_spmd

    nc = _get_nc(reps, body_mult, mode)
    in_maps = []
    for i in range(N_CORES):
        lo = i * BP
        in_maps.append({
            "enc": np.ascontiguousarray(encoder_outputs[lo:lo + BP]),
            "hid": np.ascontiguousarray(hidden[:, lo:lo + BP, :]),
            "msk": np.ascontiguousarray(mask[lo:lo + BP]),
            "w": np.ascontiguousarray(attn_w),
            "bia": np.ascontiguousarray(attn_b),
        })
    res = run_bass_kernel_spmd(nc, in_maps, list(range(N_CORES)),
                               trace=trace, **(trace_kwargs or {}))
    full = np.concatenate([res.results[i]["out"] for i in range(N_CORES)],
                          axis=0)
    return full, res


def kernel(hidden, encoder_outputs, mask, attn_w, attn_b):
    hidden = np.asarray(hidden, dtype=np.float32)
    encoder_outputs = np.asarray(encoder_outputs, dtype=np.float32)
    mask = np.asarray(mask, dtype=np.float32)
    attn_w = np.asarray(attn_w, dtype=np.float32)
    attn_b = np.asarray(attn_b, dtype=np.float32)
    full, _ = _run(hidden, encoder_outputs, mask, attn_w, attn_b)
    return full

